# revision 13
# baseline (speedup 1.0000x reference)
"""Bergman matrix layer TRN2 kernel (per-core program, batch-sharded).

v2: chunked-parallel scan with warmup restarts + sign fixup.

Per core: hidden_T fp16 [1024,T] -> out fp16 [T,1024].
  m = hidden @ W_mat + b_mat            (TensorE fp16 -> fp32 psum, 1 cpr)
  normalize per (t,h) via ACT square-accum + sqrt/recip + scale-copy
  diff trick (Pool) -> m_dram fp16 block layout [6, 16h, 512t, 256]
    (blocks 0/5 are pads read as garbage warmup by chunk 0 of each dir)
  scan: lanes p = d*64 + c*16 + h; 768 serial DVE steps; every chunk c
    reads t = 512c-256+j (lr) / 2303-512c-j (rl, heads reversed);
    chunk 0 lanes are reset to the exact init state at j=256.
  warmup convergence fixes direction; sign fixed via boundary dots
    (own j=255 tail vs predecessor j=767 tail) + prefix product.
  emission: diff tails -> v, normalize, sign, u_dram fp16
  out = gelu(x @ W_out + b_out) fp16
"""

from contextlib import ExitStack

import concourse.bass as bass
import concourse.tile as tile
from concourse import mybir
from concourse.masks import make_identity


def _register_cum_matvec():
    import numpy as np
    from concourse.dve_spec import Spec, Src0, Src1, C1, scan, AluOp, lower
    from concourse.dve_uop import DveOpSpec
    import concourse.dve_ops as dve_ops
    from concourse.dve_ops import DveOp
    for op in dve_ops.OPS:
        if op.name == "CUM_MATVEC_ANT":
            return op

    def _ref(in0, in1, s0, s1, imm2):
        p = in0.shape[0]
        a = np.asarray(in0, dtype=np.float32).reshape(p, -1)
        b = np.asarray(in1, dtype=np.float32).reshape(p, -1)
        if isinstance(s1, np.ndarray):
            s1 = s1.reshape(p, -1)
        return np.cumsum(a * b * s1, axis=1).astype(np.float32)

    spec = Spec(body=scan(AluOp.ADD, Src0 * Src1 * C1), reference=_ref)
    op = DveOp("CUM_MATVEC_ANT", spec, subdim=False, uops_sha={})
    dve_ops.OPS.append(op)
    dve_ops._SUB_OPCODE_FOR_NAME[op.name] = (
        dve_ops._CUSTOM_DVE_ROW_BASE + len(dve_ops.OPS) - 1)
    if hasattr(dve_ops, "CUSTOM_DVE_SPECS"):
        dve_ops.CUSTOM_DVE_SPECS[op.name] = op.spec
    assert max(dve_ops._SUB_OPCODE_FOR_NAME.values()) < 0x20
    for ver in ("v3", "v4"):
        uops = lower(spec, ver=ver)
        opc = dve_ops.get_dve_sub_opcode(op.name)
        op.uops_sha[ver] = DveOpSpec(
            name=op.name, opcode=opc, uops=uops, rd1_en=True).sha(ver)
    return op


CUM_MATVEC = _register_cum_matvec()

AF = mybir.ActivationFunctionType
ALU = mybir.AluOpType
F32 = mybir.dt.float32
F16 = mybir.dt.float16

HID = 1024
NH = 16
NCOLS = 4096
T = 2048
NSTEP = 768      # stream steps (512 output + 256 warmup)
CH = 16          # steps per scan tile
NCH = NSTEP // CH
RESCALE = 8192.0

BLK = 16 * 512 * 256          # elems per t-block of m_dram (h-major inside)
HST = 512 * 256               # h stride inside a block
MTOT = 6 * BLK                # blocks: [pad, t0-511, .., t1536-2047, pad]

# phase-1 production order (first-touch by the scan fronts)
TILE_ORDER = [2, 6, 10, 13, 9, 5, 3, 7, 11, 12, 8, 4, 0, 15, 1, 14]


def bcast_dim(ap, n, axis):
    dims = [list(d) for d in ap.ap]
    dims.insert(axis, [0, n])
    return bass.AP(tensor=ap.tensor, offset=ap.offset, ap=dims)


def build_kernel(ctx: ExitStack, tc: tile.TileContext):
    nc = tc.nc

    hidden_t = nc.dram_tensor("hidden_t", [HID, T], F16, kind="ExternalInput").ap()
    w_mat = nc.dram_tensor("w_mat", [HID, NCOLS], F16, kind="ExternalInput").ap()
    b_mat = nc.dram_tensor("b_mat", [1, NCOLS], F16, kind="ExternalInput").ap()
    w_out = nc.dram_tensor("w_out", [512, HID], F16, kind="ExternalInput").ap()
    b_out = nc.dram_tensor("b_out", [1, HID], F16, kind="ExternalInput").ap()
    out = nc.dram_tensor("out", [T, HID], F16, kind="ExternalOutput").ap()
    m_dram = nc.dram_tensor("m_scratch", [MTOT], F16, kind="Internal").ap()
    u_dram = nc.dram_tensor("u_scratch", [128, NSTEP * 16], F16,
                            kind="Internal").ap()

    singles = ctx.enter_context(tc.tile_pool(name="singles", bufs=1))
    w_p = ctx.enter_context(tc.tile_pool(name="wstr", bufs=2))
    mn_p = ctx.enter_context(tc.tile_pool(name="mn", bufs=2))
    sq_p = ctx.enter_context(tc.tile_pool(name="sq", bufs=2))
    st_p = ctx.enter_context(tc.tile_pool(name="st", bufs=4))
    mrec_p = ctx.enter_context(tc.tile_pool(name="mrec", bufs=3))
    mt_p = ctx.enter_context(tc.tile_pool(name="mt", bufs=2))
    scr_p = ctx.enter_context(tc.tile_pool(name="scr", bufs=2))
    sqe_p = ctx.enter_context(tc.tile_pool(name="sqe", bufs=2))
    x_p = ctx.enter_context(tc.tile_pool(name="xt", bufs=2))
    osb_p = ctx.enter_context(tc.tile_pool(name="osb", bufs=2))
    misc_p = ctx.enter_context(tc.tile_pool(name="misc", bufs=1))
    ps_mm = ctx.enter_context(tc.tile_pool(name="ps_mm", bufs=4, space="PSUM"))
    ps_out = ctx.enter_context(tc.tile_pool(name="ps_out", bufs=2, space="PSUM"))
    ps_s = ctx.enter_context(tc.tile_pool(name="ps_s", bufs=1, space="PSUM"))

    # --- singles ----------------------------------------------------------
    ident = singles.tile([128, 128], F32)
    make_identity(nc, ident)
    ones_row = singles.tile([1, 128], F16)
    nc.vector.memset(ones_row, 1.0)
    bmat_sb = singles.tile([1, NCOLS], F16)
    nc.sync.dma_start(out=bmat_sb, in_=b_mat)
    bout_sb = singles.tile([1, HID], F16)
    nc.sync.dma_start(out=bout_sb, in_=b_out)
    wout_sb = singles.tile([128, 4, HID], F16)
    nc.sync.dma_start(out=wout_sb,
                      in_=w_out.rearrange("(kt p) n -> p kt n", kt=4))
    hidT = singles.tile([128, 8, T], F16)
    nc.sync.dma_start(out=hidT,
                      in_=hidden_t.rearrange("(kc p) t -> p kc t", kc=8))
    w_init = singles.tile([128, 16], F32)
    nc.vector.memset(w_init, 1.0)
    u2 = singles.tile([128, NSTEP, 16], F16)
    rsq = singles.tile([128, NSTEP], F32)
    btile = singles.tile([128, 3, 16], F32)
    sgn = singles.tile([128, 1], F32)
    # fill the two pad blocks with 1.0 so chunk-0 garbage warmup is benign
    zsb = singles.tile([128, 8192], F16)
    nc.vector.memset(zsb, 1.0)
    for blk in (0, 5):
        for half in range(2):
            off = blk * BLK + half * (BLK // 2)
            dst = bass.AP(tensor=m_dram.tensor, offset=off,
                          ap=[[8192, 128], [1, 8192]])
            nc.sync.dma_start(out=dst, in_=zsb)

    # --- phase 1 ----------------------------------------------------------
    def phase1_tile(tt, wcol, cg):
        ps = ps_mm.tile([128, 512], F32, tag="mm")
        for kc in range(8):
            nc.tensor.matmul(ps, hidT[:, kc, tt * 128:(tt + 1) * 128],
                             wcol[:, kc, :], start=(kc == 0), stop=False)
        nc.tensor.matmul(ps, ones_row, bmat_sb[:, cg * 512:(cg + 1) * 512],
                         start=False, stop=True)
        st = st_p.tile([128, 2], F32, tag="st")
        for hh in range(2):
            scr = sq_p.tile([128, 256], F32, tag="sq")
            nc.scalar.activation(scr, ps[:, hh * 256:(hh + 1) * 256],
                                 AF.Square, accum_out=st[:, hh:hh + 1])
        # st <- 4/sqrt(S): sqrt(S/16) = sqrt(S)/4, then reciprocal
        nc.scalar.activation(st, st, AF.Sqrt, scale=1.0 / 16.0)
        nc.vector.reciprocal(st, st)
        mn = mn_p.tile([128, 2, 256], F16, tag="mn")
        for hh in range(2):
            nc.scalar.activation(mn[:, hh, :], ps[:, hh * 256:(hh + 1) * 256],
                                 AF.Copy, scale=st[:, hh:hh + 1])
        return mn

    def phase1_diff(tt, cg, mn):
        mrec = mrec_p.tile([128, 2, 256], F16, tag="mrec")
        nc.gpsimd.tensor_tensor(mrec[:, :, 0:255], mn[:, :, 0:255],
                                mn[:, :, 1:256], op=ALU.subtract)
        mn_t = mn.rearrange("p h (i j) -> p h i j", j=16)
        dst_t = mrec.rearrange("p h (i j) -> p h i j", j=16)
        nc.gpsimd.tensor_copy(dst_t[:, :, :, 15], mn_t[:, :, :, 15])
        # write into block layout: [1+tt//4, h, tw, x]
        off = ((tt // 4) + 1) * BLK + (cg * 2) * HST + (tt % 4) * 128 * 256
        dst = bass.AP(tensor=m_dram.tensor, offset=off,
                      ap=[[256, 128], [HST, 2], [1, 256]])
        nc.scalar.dma_start(out=dst, in_=mrec.transpose([0, 1, 2]))

    def phase1_group(tiles):
        for cg in range(8):
            wcol = w_p.tile([128, 8, 512], F16, tag="w")
            nc.sync.dma_start(
                out=wcol,
                in_=w_mat[:, cg * 512:(cg + 1) * 512].rearrange(
                    "(kc p) n -> p kc n", kc=8))
            for tt in tiles:
                mn = phase1_tile(tt, wcol, cg)
                phase1_diff(tt, cg, mn)

    # --- phase 2: scan ----------------------------------------------------
    def mscan_chunk(cc):
        """mt [128, CH, 256] fp16; p = d*64 + c*16 + h (rl heads reversed)."""
        j0 = cc * CH
        mtile = mt_p.tile([128, CH, 256], F16, tag="mt")
        # lr: lane (c,h) reads block c + [?]: addr = c*BLK + h*HST + tw*256
        if j0 < 256:
            lr_off, tw_step = (256 + j0) * 256, 256
        else:
            lr_off, tw_step = BLK + (j0 - 256) * 256, 256
        ap_lr = bass.AP(tensor=m_dram.tensor, offset=lr_off,
                        ap=[[HST, 64], [tw_step, CH], [1, 256]])
        nc.sync.dma_start(out=mtile[0:64], in_=ap_lr)
        # rl: lane (c,h') head-reversed, descending blocks
        if j0 < 256:
            rl_off = 5 * BLK + 15 * HST + (255 - j0) * 256
        else:
            rl_off = 4 * BLK + 15 * HST + (511 - (j0 - 256)) * 256
        ap_rl = bass.AP(tensor=m_dram.tensor, offset=rl_off,
                        ap=[[-HST, 64], [-256, CH], [1, 256]])
        nc.sync.dma_start(out=mtile[64:128], in_=ap_rl)
        return mtile

    def tail_view(scr_ap):
        dims = [list(d) for d in scr_ap.ap]
        step = dims[-1][0]
        return bass.AP(tensor=scr_ap.tensor, offset=scr_ap.offset + 15 * step,
                       ap=dims[:-1] + [[16 * step, 16]])

    state = {}

    def scan_chunk(cc):
        mtile = mscan_chunk(cc)
        scr = scr_p.tile([128, CH, 256], F32, tag="scr")
        for jj in range(CH):
            j = cc * CH + jj
            if j == 0:
                wb = bcast_dim(w_init, 16, 1)
            else:
                prev = state["prev"][:, CH - 1, :] if jj == 0 else scr[:, jj - 1, :]
                tv = tail_view(prev)
                if j in (256, 512):
                    # rescale carried state in place (avoid underflow)
                    nc.vector.tensor_scalar_mul(tv, tv, RESCALE)
                if j == 256:
                    # chunk-0 lanes start their exact chains here
                    nc.vector.memset(tail_view(prev[0:16, :]), 1.0)
                    nc.vector.memset(tail_view(prev[64:80, :]), 1.0)
                wb = bcast_dim(tv, 16, 1)
            nc.vector._custom_dve(
                CUM_MATVEC,
                out=scr[:, jj, :].rearrange("p (i x) -> p i x", i=16),
                in0=mtile[:, jj, :].rearrange("p (i x) -> p i x", i=16),
                in1=wb, s1=1.0)
        state["prev"] = scr
        emit_chunk(cc, scr)

    def emit_chunk(cc, scr):
        jb = slice(cc * CH, (cc + 1) * CH)
        tails = tail_view(scr)           # [128, CH, 16] stride-16 fp32
        nc.gpsimd.tensor_tensor(u2[:, jb, 1:16], tails[:, :, 1:16],
                                tails[:, :, 0:15], op=ALU.subtract)
        nc.gpsimd.tensor_copy(u2[:, jb, 0:1], tails[:, :, 0:1])
        if cc in (15, 31, 47):           # j = 255 / 511 / 767 boundary states
            k = {15: 0, 31: 1, 47: 2}[cc]
            nc.gpsimd.tensor_copy(btile[:, k, :], tails[:, CH - 1, :])
        sq = sqe_p.tile([128, CH, 16], F32, tag="sqe")
        nc.gpsimd.tensor_tensor(sq, u2[:, jb, :], u2[:, jb, :], op=ALU.mult)
        nc.vector.tensor_reduce(rsq[:, jb], sq, axis=mybir.AxisListType.X,
                                op=ALU.add)
        nc.scalar.activation(rsq[:, jb], rsq[:, jb], AF.Sqrt)
        nc.vector.reciprocal(rsq[:, jb], rsq[:, jb])
        nc.gpsimd.tensor_tensor(u2[:, jb, :], u2[:, jb, :],
                                bcast_dim(rsq[:, jb], 16, 2), op=ALU.mult)

    # --- sign fixup -------------------------------------------------------
    def sign_fix():
        bsh = misc_p.tile([128, 16], F32)
        # B: predecessor lane's j=767 tail, shifted 16 partitions up (c-1)
        nc.sync.dma_start(out=bsh[16:128, :], in_=btile[0:112, 2, :])
        prod = misc_p.tile([128, 16], F32)
        nc.vector.tensor_tensor(prod, btile[:, 0, :], bsh, op=ALU.mult)
        r = misc_p.tile([128, 1], F32)
        nc.vector.tensor_reduce(r, prod, axis=mybir.AxisListType.X, op=ALU.add)
        ge = misc_p.tile([128, 1], F32)
        nc.vector.tensor_scalar(ge, r, 0.0, None, op0=ALU.is_ge)
        nc.vector.tensor_scalar(ge, ge, 2.0, 1.0, op0=ALU.mult,
                                op1=ALU.subtract)
        ps1 = ps_s.tile([128, 128], F32, tag="pss")
        nc.tensor.transpose(ps1[0:1, :], ge, ident)
        sT = misc_p.tile([1, 128], F32)
        nc.scalar.activation(sT, ps1[0:1, :], AF.Copy)
        # chunk-0 lanes (p % 64 < 16) -> +1
        c0v = bass.AP(tensor=sT.tensor, offset=sT.offset,
                      ap=[list(sT.ap[0]), [64, 2], [1, 16]])
        nc.vector.memset(c0v, 1.0)
        sTv = sT.rearrange("q (d c h) -> q d c h", d=2, c=4)
        nc.vector.tensor_tensor(sTv[:, :, 2, :], sTv[:, :, 2, :],
                                sTv[:, :, 1, :], op=ALU.mult)
        nc.vector.tensor_tensor(sTv[:, :, 3, :], sTv[:, :, 3, :],
                                sTv[:, :, 2, :], op=ALU.mult)
        ps2 = ps_s.tile([128, 128], F32, tag="pss2")
        nc.tensor.transpose(ps2[:, 0:1], sT, ident[0:1, 0:1])
        nc.scalar.activation(sgn, ps2[:, 0:1], AF.Copy)
        u2f = u2.rearrange("p j i -> p (j i)")
        nc.vector.tensor_scalar_mul(u2f, u2f, sgn)
        nc.sync.dma_start(out=u_dram, in_=u2f)

    # --- phase 3: out blocks ---------------------------------------------
    def out_block(b):
        t0 = b * 128
        lc, lj0 = b // 4, 256 + 128 * (b % 4)
        rc = 3 - b // 4
        rj0 = 2303 - 512 * rc - t0
        xk = x_p.tile([128, 4, 128], F16, tag="xk")
        xv = xk.rearrange("t kt (h2 d i) -> t (kt h2) d i", d=2, i=16)
        US = NSTEP * 16
        ap_lr = bass.AP(tensor=u_dram.tensor,
                        offset=(lc * 16) * US + lj0 * 16,
                        ap=[[16, 128], [16 * US, 16], [1, 16]])
        nc.sync.dma_start(out=xv[:, :, 0, :], in_=ap_lr)
        ap_rl = bass.AP(tensor=u_dram.tensor,
                        offset=(64 + rc * 16 + 15) * US + rj0 * 16,
                        ap=[[-16, 128], [-16 * US, 16], [1, 16]])
        nc.sync.dma_start(out=xv[:, :, 1, :], in_=ap_rl)
        osb = osb_p.tile([128, 2, 512], F16, tag="osb")
        for oc in range(2):
            ps = ps_out.tile([128, 512], F32, tag="po")
            for kt in range(4):
                nc.tensor.matmul(ps, xk[:, kt, :],
                                 wout_sb[:, kt, oc * 512:(oc + 1) * 512],
                                 start=(kt == 0), stop=False)
            nc.tensor.matmul(ps, ones_row, bout_sb[:, oc * 512:(oc + 1) * 512],
                             start=False, stop=True)
            nc.scalar.activation(osb[:, oc, :], ps, AF.Gelu)
        nc.sync.dma_start(out=out[b * 128:(b + 1) * 128, :],
                          in_=osb.rearrange("p a b -> p (a b)"))

    # --- schedule ---------------------------------------------------------
    phase1_group(TILE_ORDER[:8])
    phase1_group(TILE_ORDER[8:])
    for cc in range(NCH):
        scan_chunk(cc)
    sign_fix()
    for b in range(16):
        out_block(b)


def build_nc():
    import concourse.bacc as bacc
    nc = bacc.Bacc("TRN2", target_bir_lowering=False, debug=False)
    with tile.TileContext(nc) as tc:
        with ExitStack() as ctx:
            build_kernel(ctx, tc)
    nc.compile()
    return nc


# ----------------------------------------------------------------------------
# Self-contained entry point: full inputs in, full outputs out (8 cores).
# ----------------------------------------------------------------------------
import numpy as np

_NC_CACHE = {}


def _get_nc(T_=2048):
    if T_ not in _NC_CACHE:
        _NC_CACHE[T_] = build_nc()
    return _NC_CACHE[T_]


def kernel(hidden_states, W_mat, b_mat, W_out, b_out):
    from concourse.bass_utils import run_bass_kernel_spmd
    B = hidden_states.shape[0]
    nc = _get_nc(2048)
    w_mat = np.ascontiguousarray(W_mat, dtype=np.float16)
    b_mat_ = np.ascontiguousarray(b_mat, dtype=np.float16).reshape(1, -1)
    w_out = np.ascontiguousarray(W_out, dtype=np.float16)
    b_out_ = np.ascontiguousarray(b_out, dtype=np.float16).reshape(1, -1)
    in_maps = [
        {
            "hidden_t": np.ascontiguousarray(
                np.asarray(hidden_states[b], dtype=np.float16).T),
            "w_mat": w_mat,
            "b_mat": b_mat_,
            "w_out": w_out,
            "b_out": b_out_,
        }
        for b in range(B)
    ]
    res = run_bass_kernel_spmd(nc, in_maps, list(range(B)))
    return np.stack(
        [res.results[b]["out"].astype(np.float32) for b in range(B)], axis=0)


# revision 43
# speedup vs baseline: 1.1322x; 1.1322x over previous
"""Bergman matrix layer TRN2 kernel (per-core program, batch-sharded).

v3: chunked-parallel scan, 512-step warmup, fp32 m path (f32r matmuls).

Per core: hidden_T fp32 [1024,T] -> out fp16 [T,1024].
  m = hidden @ W_mat + b_mat            (TensorE f32r -> fp32 psum, 1 cpr)
  normalize per (t,h) via ACT square-accum + sqrt/recip + scale-copy
  diff trick (Pool) -> m_dram fp32 [2048, 16h, 256]  (block-foldable)
  scan: lanes: lr p = c*16+h, rl p = 64+(3-c)*16+h; 1024 DVE steps;
    lr chunk c reads t = 512(c-1)+j ; rl chunk c reads t = 2559-512c-j.
    chunk-0 lanes idle on stale data for j<512, reset to exact init at
    j=512 (so every output has >=512 warmup steps or an exact start).
  sign fixed via boundary dots (own j=511 tail vs pred j=1023 tail)
    + prefix product over chunks.
  emission: diff tails -> v fp16, normalize, sign, u_dram fp16
  out = gelu(x @ W_out + b_out) fp16   (fp16 matmul)
"""

from contextlib import ExitStack

import concourse.bass as bass
import concourse.tile as tile
from concourse import mybir
from concourse.masks import make_identity


def _register_cum_matvec():
    import numpy as np
    from concourse.dve_spec import Spec, Src0, Src1, C1, scan, AluOp, lower
    from concourse.dve_uop import DveOpSpec
    import concourse.dve_ops as dve_ops
    from concourse.dve_ops import DveOp
    for op in dve_ops.OPS:
        if op.name == "CUM_MATVEC_ANT":
            return op

    def _ref(in0, in1, s0, s1, imm2):
        p = in0.shape[0]
        a = np.asarray(in0, dtype=np.float32).reshape(p, -1)
        b = np.asarray(in1, dtype=np.float32).reshape(p, -1)
        if isinstance(s1, np.ndarray):
            s1 = s1.reshape(p, -1)
        return np.cumsum(a * b * s1, axis=1).astype(np.float32)

    spec = Spec(body=scan(AluOp.ADD, Src0 * Src1 * C1), reference=_ref)
    op = DveOp("CUM_MATVEC_ANT", spec, subdim=False, uops_sha={})
    dve_ops.OPS.append(op)
    dve_ops._SUB_OPCODE_FOR_NAME[op.name] = (
        dve_ops._CUSTOM_DVE_ROW_BASE + len(dve_ops.OPS) - 1)
    if hasattr(dve_ops, "CUSTOM_DVE_SPECS"):
        dve_ops.CUSTOM_DVE_SPECS[op.name] = op.spec
    assert max(dve_ops._SUB_OPCODE_FOR_NAME.values()) < 0x20
    for ver in ("v3", "v4"):
        uops = lower(spec, ver=ver)
        opc = dve_ops.get_dve_sub_opcode(op.name)
        op.uops_sha[ver] = DveOpSpec(
            name=op.name, opcode=opc, uops=uops, rd1_en=True).sha(ver)
    return op


CUM_MATVEC = _register_cum_matvec()

AF = mybir.ActivationFunctionType
ALU = mybir.AluOpType
F32 = mybir.dt.float32
F16 = mybir.dt.float16
F32R = mybir.dt.float32r

HID = 1024
NH = 16
NCOLS = 4096
T = 2048
NSTEP = 1024     # stream steps (512 output + 512 warmup)
CH = 16          # steps per scan tile
NCH = NSTEP // CH
RESCALE = 8192.0

HST = 512 * 256               # h stride inside a 512-row block (elems)
BLK = 16 * HST                # elems per t-block of m_dram


def _tile_order():
    """First-touch order of 128-row tiles by the 6 scan fronts."""
    seen, order = set(), []
    for j in range(NSTEP):
        ts = [512 * c + j for c in (0, 1, 2)] + \
             [2047 - 512 * c - j for c in (0, 1, 2)]
        for t in ts:
            if 0 <= t < T and (t // 128) not in seen:
                seen.add(t // 128)
                order.append(t // 128)
    for k in range(16):
        if k not in seen:
            order.append(k)
    return order


TILE_ORDER = _tile_order()


def bcast_dim(ap, n, axis):
    dims = [list(d) for d in ap.ap]
    dims.insert(axis, [0, n])
    return bass.AP(tensor=ap.tensor, offset=ap.offset, ap=dims)


def build_kernel(ctx: ExitStack, tc: tile.TileContext):
    nc = tc.nc

    hidden_t = nc.dram_tensor("hidden_t", [HID, T], F32, kind="ExternalInput").ap()
    w_mat = nc.dram_tensor("w_mat", [HID, NCOLS], F32, kind="ExternalInput").ap()
    b_mat = nc.dram_tensor("b_mat", [1, NCOLS], F32, kind="ExternalInput").ap()
    w_out = nc.dram_tensor("w_out", [512, HID], F16, kind="ExternalInput").ap()
    b_out = nc.dram_tensor("b_out", [1, HID], F16, kind="ExternalInput").ap()
    out = nc.dram_tensor("out", [T, HID], F16, kind="ExternalOutput").ap()
    m_dram = nc.dram_tensor("m_scratch", [4 * BLK], F32, kind="Internal").ap()
    u_dram = nc.dram_tensor("u_scratch", [96 * 16 * NSTEP], F16,
                            kind="Internal").ap()

    singles = ctx.enter_context(tc.tile_pool(name="singles", bufs=1))
    ht_p = ctx.enter_context(tc.tile_pool(name="ht", bufs=8))
    w_p = ctx.enter_context(tc.tile_pool(name="wstr", bufs=2))
    st_p = ctx.enter_context(tc.tile_pool(name="st", bufs=4))
    mrec_p = ctx.enter_context(tc.tile_pool(name="mrec", bufs=3))
    mt_p = ctx.enter_context(tc.tile_pool(name="mt", bufs=2))
    scr_p = ctx.enter_context(tc.tile_pool(name="scr", bufs=2))
    sqe_p = ctx.enter_context(tc.tile_pool(name="sqe", bufs=2))
    x_p = ctx.enter_context(tc.tile_pool(name="xt", bufs=2))
    osb_p = ctx.enter_context(tc.tile_pool(name="osb", bufs=2))
    misc_p = ctx.enter_context(tc.tile_pool(name="misc", bufs=1))
    ps_mm = ctx.enter_context(tc.tile_pool(name="ps_mm", bufs=4, space="PSUM"))
    ps_out = ctx.enter_context(tc.tile_pool(name="ps_out", bufs=2, space="PSUM"))
    ps_s = ctx.enter_context(tc.tile_pool(name="ps_s", bufs=1, space="PSUM"))

    # --- singles ----------------------------------------------------------
    ident = singles.tile([128, 128], F32)
    make_identity(nc, ident)
    ones_r = singles.tile([1, 128], F32)
    nc.vector.memset(ones_r, 1.0)
    ones_h = singles.tile([1, 128], F16)
    nc.vector.memset(ones_h, 1.0)
    bmat_sb = singles.tile([1, NCOLS], F32)
    nc.sync.dma_start(out=bmat_sb, in_=b_mat)
    bout_sb = singles.tile([1, HID], F16)
    nc.sync.dma_start(out=bout_sb, in_=b_out)
    wout_sb = singles.tile([128, 4, HID], F16)
    nc.sync.dma_start(out=wout_sb,
                      in_=w_out.rearrange("(kt p) n -> p kt n", kt=4))
    w_init = singles.tile([128, 16], F32)
    nc.vector.memset(w_init, 1.0)
    u2 = singles.tile([96, 16, NSTEP], F16)   # i-major: free = (i, j)
    btile = singles.tile([96, 2, 16], F32)   # tails at j=511 (A), j=1023 (B)
    sgn = singles.tile([96, 1], F32)

    # --- phase 1: matmul with host-prediffed W -> psum -> m_dram ---------
    def phase1_tile(tt, ht, wcol, cg):
        ps = ps_mm.tile([128, 512], F32, tag="mm")
        for kc in range(8):
            nc.tensor.matmul(ps, ht[:, kc, :], wcol[:, kc, :],
                             start=(kc == 0), stop=False)
        nc.tensor.matmul(ps, ones_r, bmat_sb[:, cg * 512:(cg + 1) * 512],
                         start=False, stop=True)
        mrec = mrec_p.tile([128, 512], F32, tag="mrec")
        nc.scalar.activation(mrec, ps, AF.Copy)
        # m_dram records: addr = (t>>9)*BLK + h*HST + (t&511)*256
        off = (tt // 4) * BLK + (cg * 2) * HST + (tt % 4) * 128 * 256
        dst = bass.AP(tensor=m_dram.tensor, offset=off,
                      ap=[[256, 128], [HST, 2], [1, 256]])
        nc.scalar.dma_start(out=dst, in_=mrec.rearrange("p (h x) -> p h x", h=2))

    def phase1_group(tiles):
        hts = {}
        for cg in range(8):
            wcol = w_p.tile([128, 8, 512], F32, tag="w")
            nc.sync.dma_start(
                out=wcol,
                in_=w_mat[:, cg * 512:(cg + 1) * 512].rearrange(
                    "(kc p) n -> p kc n", kc=8))
            for tt in tiles:
                if cg == 0:
                    ht = ht_p.tile([128, 8, 128], F32, tag="ht",
                                   name=f"ht{tt}")
                    nc.sync.dma_start(
                        out=ht,
                        in_=hidden_t[:, tt * 128:(tt + 1) * 128].rearrange(
                            "(kc p) t -> p kc t", kc=8))
                    hts[tt] = ht
                phase1_tile(tt, hts[tt], wcol, cg)

    # --- phase 2: scan ----------------------------------------------------
    def mscan_chunk(cc):
        """mt [128, CH, 256] fp32; lr p=c*16+h (c<3), rl p=64+(2-c)*16+h."""
        j0 = cc * CH
        mtile = mt_p.tile([96, CH, 256], F32, tag="mt")
        if j0 < 512:
            lr_off = j0 * 256
            rl_off = BLK + (511 - j0) * 256
        else:
            lr_off = BLK + (j0 - 512) * 256
            rl_off = (1023 - j0) * 256
        ap_lr = bass.AP(tensor=m_dram.tensor, offset=lr_off,
                        ap=[[HST, 48], [256, CH], [1, 256]])
        nc.gpsimd.dma_start(out=mtile[0:48], in_=ap_lr)
        ap_rl = bass.AP(tensor=m_dram.tensor, offset=rl_off,
                        ap=[[HST, 48], [-256, CH], [1, 256]])
        nc.gpsimd.dma_start(out=mtile[48:96], in_=ap_rl)
        return mtile

    def tail_view(scr_ap):
        dims = [list(d) for d in scr_ap.ap]
        step = dims[-1][0]
        return bass.AP(tensor=scr_ap.tensor, offset=scr_ap.offset + 15 * step,
                       ap=dims[:-1] + [[16 * step, 16]])

    state = {}

    def scan_chunk(cc):
        mtile = mscan_chunk(cc)
        scr = scr_p.tile([96, CH, 256], F32, tag="scr")
        for jj in range(CH):
            j = cc * CH + jj
            if j == 0:
                wb = bcast_dim(w_init[0:96, :], 16, 1)
            else:
                prev = state["prev"][:, CH - 1, :] if jj == 0 else scr[:, jj - 1, :]
                tv = tail_view(prev)
                if j in (256, 512, 768):
                    # rescale carried state in place (avoid underflow)
                    nc.vector.tensor_scalar_mul(tv, tv, RESCALE)
                wb = bcast_dim(tv, 16, 1)
            nc.vector._custom_dve(
                CUM_MATVEC,
                out=scr[:, jj, :].rearrange("p (i x) -> p i x", i=16),
                in0=mtile[:, jj, :].rearrange("p (i x) -> p i x", i=16),
                in1=wb, s1=1.0)
        state["prev"] = scr
        emit_chunk(cc, scr)

    def emit_chunk(cc, scr):
        jb = slice(cc * CH, (cc + 1) * CH)
        tails = tail_view(scr)           # [128, CH, 16] stride-16 fp32
        dtmp = sqe_p.tile([96, CH, 16], F32, tag="dtmp")
        nc.gpsimd.tensor_tensor(dtmp[:, :, 1:16], tails[:, :, 1:16],
                                tails[:, :, 0:15], op=ALU.subtract)
        nc.gpsimd.tensor_copy(dtmp[:, :, 0:1], tails[:, :, 0:1])
        if cc in (31, 63):               # j = 511 / 1023 boundary states
            nc.gpsimd.tensor_copy(btile[:, {31: 0, 63: 1}[cc], :],
                                  tails[:, CH - 1, :])
        sq = sqe_p.tile([96, CH, 16], F32, tag="sqe")
        nc.gpsimd.tensor_tensor(sq, dtmp, dtmp, op=ALU.mult)
        rsq = st_p.tile([96, CH], F32, tag="rsq")
        nc.vector.tensor_reduce(rsq, sq, axis=mybir.AxisListType.X, op=ALU.add)
        nc.scalar.activation(rsq, rsq, AF.Sqrt)
        nc.vector.reciprocal(rsq, rsq)
        # write only the NORMALIZED values (O(1)) into fp16 u2 (i-major,
        # iterated (j, i) to match dtmp). rl half goes in j-reversed.
        # Partition windows must be 32-aligned: the rl op covers [32:96]
        # (garbage into lr lanes 32-47), then the lr op [0:48] fixes them.
        j0 = cc * CH
        j0r = NSTEP - 1 - j0
        for p0, p1 in ((32, 64), (64, 96)):
            rl_base = u2[p0:p1]
            rl_out = bass.AP(tensor=u2.tensor, offset=rl_base.offset + j0r,
                             ap=[list(rl_base.ap[0]), [-1, CH], [NSTEP, 16]])
            nc.gpsimd.tensor_tensor(rl_out, dtmp[p0:p1],
                                    bcast_dim(rsq[p0:p1], 16, 2), op=ALU.mult)
        lr_out = bass.AP(tensor=u2.tensor, offset=u2.offset + j0,
                         ap=[list(u2[0:48].ap[0]), [1, CH], [NSTEP, 16]])
        nc.gpsimd.tensor_tensor(lr_out, dtmp[0:48],
                                bcast_dim(rsq[0:48], 16, 2), op=ALU.mult)

    # --- sign fixup -------------------------------------------------------
    def sign_fix():
        bsh = misc_p.tile([96, 16], F32)
        nc.vector.memset(bsh, 1.0)
        # predecessor (c-1) tail at j=1023: lr lanes p-16, rl lanes p+16
        nc.sync.dma_start(out=bsh[16:48, :], in_=btile[0:32, 1, :])
        nc.sync.dma_start(out=bsh[48:80, :], in_=btile[64:96, 1, :])
        prod = misc_p.tile([96, 16], F32)
        nc.vector.tensor_tensor(prod, btile[:, 0, :], bsh, op=ALU.mult)
        r = misc_p.tile([96, 1], F32)
        nc.vector.tensor_reduce(r, prod, axis=mybir.AxisListType.X, op=ALU.add)
        ge = misc_p.tile([96, 1], F32)
        nc.vector.tensor_scalar(ge, r, 0.0, None, op0=ALU.is_ge)
        nc.vector.tensor_scalar(ge, ge, 2.0, 1.0, op0=ALU.mult,
                                op1=ALU.subtract)
        ps1 = ps_s.tile([128, 128], F32, tag="pss")
        nc.tensor.transpose(ps1[0:1, 0:96], ge, ident[0:96, 0:96])
        sT = misc_p.tile([1, 96], F32)
        nc.scalar.activation(sT, ps1[0:1, 0:96], AF.Copy)
        # chunk-0 lanes (lr p<16, rl p in [80,96)) -> +1
        c0v = bass.AP(tensor=sT.tensor, offset=sT.offset,
                      ap=[list(sT.ap[0]), [80, 2], [1, 16]])
        nc.vector.memset(c0v, 1.0)
        # prefix products: lr c ascending 16/32; rl descending 64/48
        for dst, src_ in ((32, 16), (48, 64)):
            nc.vector.tensor_tensor(sT[:, dst:dst + 16], sT[:, dst:dst + 16],
                                    sT[:, src_:src_ + 16], op=ALU.mult)
        ps2 = ps_s.tile([128, 128], F32, tag="pss2")
        nc.tensor.transpose(ps2[0:96, 0:1], sT, ident[0:1, 0:1])
        nc.scalar.activation(sgn, ps2[0:96, 0:1], AF.Copy)
        u2f = u2.rearrange("p a b -> p (a b)")
        nc.vector.tensor_scalar_mul(u2f, u2f, sgn)
        # store [lane, i, s] straight (u2 is already i-major)
        u_dst = bass.AP(tensor=u_dram.tensor, offset=0,
                        ap=[[16 * NSTEP, 96], [NSTEP, 16], [1, NSTEP]])
        nc.sync.dma_start(out=u_dst, in_=u2)

    # --- phase 3: out blocks ---------------------------------------------
    def out_block(b):
        t0 = b * 128
        lc = max(0, b // 4 - 1)       # lr chunk: c0 covers t<1024
        lj0 = t0 - 512 * lc
        rc = max(0, 2 - b // 4)       # rl chunk: c0 covers t>=1024
        rs0 = t0 + 512 * rc - 1024    # rl stored index (j-reversed store)
        xk = x_p.tile([128, 4, 128], F16, tag="xk")
        # partition p = x-col within kt (h2*16+i); u addr = lane*16K + i*1K + s
        US = NSTEP
        ap_lr = bass.AP(tensor=u_dram.tensor,
                        offset=(lc * 16) * 16 * US + lj0,
                        ap=[[US, 64], [4 * 16 * US, 4], [1, 128]])
        nc.sync.dma_start(out=xk[0:64], in_=ap_lr)
        ap_rl = bass.AP(tensor=u_dram.tensor,
                        offset=(48 + (2 - rc) * 16) * 16 * US + rs0,
                        ap=[[US, 64], [4 * 16 * US, 4], [1, 128]])
        nc.sync.dma_start(out=xk[64:128], in_=ap_rl)
        osb = osb_p.tile([128, 2, 512], F16, tag="osb")
        for oc in range(2):
            ps = ps_out.tile([128, 512], F32, tag="po")
            for kt in range(4):
                nc.tensor.matmul(ps, xk[:, kt, :],
                                 wout_sb[:, kt, oc * 512:(oc + 1) * 512],
                                 start=(kt == 0), stop=False)
            nc.tensor.matmul(ps, ones_h, bout_sb[:, oc * 512:(oc + 1) * 512],
                             start=False, stop=True)
            nc.scalar.activation(osb[:, oc, :], ps, AF.Gelu)
        nc.sync.dma_start(out=out[b * 128:(b + 1) * 128, :],
                          in_=osb.rearrange("p a b -> p (a b)"))

    # --- schedule ---------------------------------------------------------
    phase1_group(TILE_ORDER[:8])
    for cc in range(8):
        scan_chunk(cc)
    phase1_group(TILE_ORDER[8:])
    for cc in range(8, NCH):
        scan_chunk(cc)
    sign_fix()
    for b in range(16):
        out_block(b)


def build_nc():
    import concourse.bacc as bacc
    nc = bacc.Bacc("TRN2", target_bir_lowering=False, debug=False)
    with tile.TileContext(nc) as tc:
        with ExitStack() as ctx:
            build_kernel(ctx, tc)
    nc.compile()
    return nc


# ----------------------------------------------------------------------------
# Self-contained entry point: full inputs in, full outputs out (8 cores).
# ----------------------------------------------------------------------------
import numpy as np

_NC_CACHE = {}


def _get_nc(T_=2048):
    if T_ not in _NC_CACHE:
        _NC_CACHE[T_] = build_nc()
    return _NC_CACHE[T_]


def kernel(hidden_states, W_mat, b_mat, W_out, b_out):
    from concourse.bass_utils import run_bass_kernel_spmd
    B = hidden_states.shape[0]
    nc = _get_nc(2048)
    W32 = np.asarray(W_mat, dtype=np.float32)
    b32 = np.asarray(b_mat, dtype=np.float32).reshape(-1)
    # fold the column-difference trick and a constant normalizer into W/b
    sig = np.sqrt((W32.astype(np.float64) ** 2).mean() * W32.shape[0])
    scale = np.float32(4.0 / (16.0 * sig))
    Wd = W32.reshape(HID, 256, 16).copy()
    Wd[:, :, :15] -= Wd[:, :, 1:]
    bd = b32.reshape(256, 16).copy()
    bd[:, :15] -= bd[:, 1:]
    w_mat = np.ascontiguousarray(Wd.reshape(HID, NCOLS) * scale)
    b_mat_ = np.ascontiguousarray(bd.reshape(1, NCOLS) * scale)
    # permute W_out rows: new row kt*128 + d*64 + h2*16 + i
    #                   = old row (kt*4+h2)*32 + d*16 + i
    kt, d, h2, i = np.meshgrid(np.arange(4), np.arange(2), np.arange(4),
                               np.arange(16), indexing="ij")
    newrow = (kt * 128 + d * 64 + h2 * 16 + i).reshape(-1)
    oldrow = ((kt * 4 + h2) * 32 + d * 16 + i).reshape(-1)
    perm = np.empty(512, np.int64)
    perm[newrow] = oldrow
    w_out = np.ascontiguousarray(
        np.asarray(W_out, dtype=np.float16)[perm])
    b_out_ = np.ascontiguousarray(b_out, dtype=np.float16).reshape(1, -1)
    in_maps = [
        {
            "hidden_t": np.ascontiguousarray(
                np.asarray(hidden_states[b], dtype=np.float32).T),
            "w_mat": w_mat,
            "b_mat": b_mat_,
            "w_out": w_out,
            "b_out": b_out_,
        }
        for b in range(B)
    ]
    res = run_bass_kernel_spmd(nc, in_maps, list(range(B)))
    return np.stack(
        [res.results[b]["out"].astype(np.float32) for b in range(B)], axis=0)
